# revision 14
# baseline (speedup 1.0000x reference)
"""Trainium2 Bass kernel for a causal AttentionBlock (dense transformer).

Model (reference):
    qkv = x @ Wqkv + bqkv ; 16-head causal attention (no out-proj)
    x2  = x + attn_out
    out = x2 + relu(x2 @ W1 + b1) @ W2 + b2

x: [2, 2048, 1024] fp32. 8 NeuronCores.

Sharding (no collectives — on-chip collectives are too slow at these sizes):
data-parallel over (batch, query-chunk). Core c handles batch b = c//4 and the
balanced causal chunk pair (j, 7-j), j = c%4, of 8x256-row chunks, giving every
core the same 512 query rows. Each core redundantly projects K/V for its whole
batch (uniform SPMD program), computes attention for its rows with a shipped
additive causal mask, then the MLP for its rows. Host concatenates.

Everything on-chip runs transposed ([feature, row] layout) so that:
  - scoresT = kT.T @ qT needs no transposes of activations,
  - softmax denominators come free via a ones-column appended to V,
  - softmax max-subtraction is skipped (scores are bounded, exp is safe),
  - per-partition biases fold into PSUM-evacuation activations.
For odd heads the ones-column precedes V ([1|v] vs [v|1]) so the AV output
lands on partitions 64..127, aligned with the head's feature slice.
"""
import os
import sys

sys.path.insert(0, "/opt/trn_rl_repo")

import numpy as np

import bass_rust
import concourse.bass as bass
import concourse.mybir as mybir
import concourse.tile as tile
from concourse.bass_utils import run_bass_kernel_spmd

# ---------------------------------------------------------------- constants
B, T, N = 2, 2048, 1024
H, D = 16, 64
NCORES = 8
CH = 256               # query chunk rows
TKA, TKB = 1024, 2048  # uniform kv extents for chunk A / chunk B
F32 = mybir.dt.float32
F32R = mybir.dt.float32r

# Matmul input dtype: float32r (TF32-like, ~1e-3 max matmul rel err, 4x faster)
# or float32 (bit-accurate, 4 cycles/row). Flip with env KERNEL_F32R=0/1.
USE_F32R = os.environ.get("KERNEL_F32R", "0") == "1"
MM = F32R if USE_F32R else F32

_prog_cache = {}


def _r(ap):
    """View an fp32 DRAM AP as float32r for DMA into float32r tiles."""
    return ap.bitcast(F32R) if USE_F32R else ap


# ------------------------------------------------------------- wait legalizer
def _legalize_waits(nc):
    """This walrus build accepts <=1 sync wait on most instructions and 0 on
    fp32/fp32r Matmult (fused self-loading LDW). Move excess waits onto bare
    EventSemaphore instructions inserted before, on the same engine."""
    n_split = 0
    for fn in nc.m.functions:
        for blk in fn.blocks:
            insts = blk.instructions
            out = []
            for inst in insts:
                si = inst.sync_info
                waits = list(si.on_wait) if si is not None else []
                tname = type(inst).__name__
                if tname in ("InstMatmult", "InstMatmultMx"):
                    maxw = 0
                    for arg in inst.ins:
                        dt = getattr(arg, "dtype", None)
                        if dt is not None and mybir.dt.size(dt) == 2:
                            maxw = 1
                            break
                else:
                    maxw = 1
                if len(waits) > maxw:
                    extra = waits[:-maxw] if maxw else waits
                    keep = waits[-maxw:] if maxw else []
                    for k, w in enumerate(extra):
                        ev = mybir.InstEventSemaphore(
                            name=f"{inst.name}-lw{k}", ins=[], outs=[]
                        )
                        ev.engine = inst.engine
                        ev.sync_info = bass_rust.SyncInfo(on_wait=[w], on_update=[])
                        out.append(ev)
                        n_split += 1
                    inst.sync_info = bass_rust.SyncInfo(
                        on_wait=keep, on_update=list(si.on_update)
                    )
                out.append(inst)
            insts[:] = out
    return n_split


# ------------------------------------------------------------------- program
def _build_program():
    nc = bass.Bass("TRN2", debug=False, num_devices=NCORES)

    t_ = {}
    t_["xb"] = nc.dram_tensor("xb", [T, N], F32, kind="ExternalInput").ap()
    t_["xq"] = nc.dram_tensor("xq", [2 * CH, N], F32, kind="ExternalInput").ap()
    t_["wqkv_t"] = nc.dram_tensor("wqkv_t", [8, 24, 128, 128], F32,
                                  kind="ExternalInput").ap()
    t_["wqkv"] = nc.dram_tensor("wqkv", [N, 3 * N], F32, kind="ExternalInput").ap()
    t_["w1_t"] = nc.dram_tensor("w1_t", [32, 8, 128, 128], F32,
                                kind="ExternalInput").ap()
    t_["w2_t"] = nc.dram_tensor("w2_t", [8, 32, 128, 128], F32,
                                kind="ExternalInput").ap()
    for nm, sz in (("bqs", N), ("bk", N), ("bv", N), ("bvo", N),
                   ("b1", 4 * N), ("b2", N)):
        t_[nm] = nc.dram_tensor(nm, [sz], F32, kind="ExternalInput").ap()
    t_["gates"] = nc.dram_tensor("gates", [128, 16], F32,
                                 kind="ExternalInput").ap()
    t_["maskd"] = nc.dram_tensor("maskd", [256, CH], F32,
                                 kind="ExternalInput").ap()
    t_["out"] = nc.dram_tensor("out", [2 * CH, N], F32, kind="ExternalOutput").ap()
    t_["kt_dram"] = nc.dram_tensor("kt_scratch", [N, T], MM).ap()

    with tile.TileContext(nc) as tc:
        _emit(nc, tc, t_)
    return nc


def _emit(nc, tc, t_):
    AF = mybir.ActivationFunctionType
    OP = mybir.AluOpType

    with tc.tile_pool(name="const", bufs=1) as const:
        ident = const.tile([128, 128], F32)
        nc.gpsimd.memset(ident[:], 0.0)
        nc.gpsimd.affine_select(
            out=ident[:], in_=ident[:], compare_op=OP.not_equal, fill=1.0,
            base=0, pattern=[[-1, 128]], channel_multiplier=1,
        )
        ones = const.tile([128, 64], F32)
        nc.vector.memset(ones[:], 1.0)
        bias = {}
        for nm, w in (("bqs", 8), ("bk", 8), ("bv", 8), ("bvo", 8),
                      ("b1", 32), ("b2", 8)):
            bias[nm] = const.tile([128, w], F32, name=f"b_{nm}")
            nc.sync.dma_start(bias[nm][:], t_[nm].rearrange("(f p) -> p f", p=128))

        with tc.tile_pool(name="x2t", bufs=8) as px2t:
            x2T = [px2t.tile([128, 2 * CH], F32, tag="x2t", name=f"x2T{i}")
                   for i in range(8)]

            with tc.tile_pool(name="keep", bufs=1) as keep:
                # v_aug[rt]: [128 kv-rows, head h -> [v|1] (even) / [1|v] (odd)]
                v_aug = [keep.tile([128, H, D + 1], MM, tag=f"va{rt}",
                                   name=f"va{rt}") for rt in range(T // 128)]
                qT = [keep.tile([128, 2 * CH], MM, tag=f"qt{f}",
                              name=f"qT{f}") for f in range(8)]
                xqT = [keep.tile([128, 2 * CH], F32, tag=f"xqt{f}",
                               name=f"xqT{f}") for f in range(8)]
                if USE_F32R:
                    xqr = [keep.tile([128, 2 * CH], F32R, tag=f"xqr{f}",
                                   name=f"xqr{f}") for f in range(8)]
                else:
                    xqr = xqT

                _phase1(nc, tc, AF, OP, t_, bias, ident, v_aug, qT, xqT, xqr)
                _phase2(nc, tc, AF, OP, t_, bias, ones, v_aug, qT, xqT, x2T)
            _phase3(nc, tc, AF, OP, t_, bias, ident, x2T, t_["out"])


def _phase1(nc, tc, AF, OP, t_, bias, ident, v_aug, qT, xqT, xqr):
    """Transposes + Q/K/V projections. kT spills to DRAM; v_aug stays in SBUF."""
    xb, xq, wqkv_t, wqkv, kt_dram = (t_["xb"], t_["xq"], t_["wqkv_t"],
                                     t_["wqkv"], t_["kt_dram"])
    with tc.tile_pool(name="p1", bufs=2) as p1, \
         tc.tile_pool(name="p1st", bufs=4) as p1st, \
         tc.tile_pool(name="p1wv", bufs=9) as p1wv, \
         tc.tile_pool(name="p1x", bufs=2) as p1x, \
         tc.tile_pool(name="ps1", bufs=2, space="PSUM") as ps1, \
         tc.tile_pool(name="ps1p", bufs=4, space="PSUM") as ps1p:

        # ones column of v_aug (all heads: [v | 1]); memset lacks an fp32r
        # encoding, so write the bits through a uint32 view
        for rt in range(T // 128):
            if MM == F32:
                nc.vector.memset(v_aug[rt][:, :, D:D + 1], 1.0)
            else:
                nc.vector.memset(
                    v_aug[rt][:, :, D:D + 1].bitcast(mybir.dt.uint32),
                    0x3F800000)

        # --- xq transpose: xqT[f] [128, 512] (+ fp32r copy for matmul use)
        for rt in range(4):
            xrow = p1.tile([128, 1024], F32, tag="xrow")
            nc.sync.dma_start(xrow[:], xq[rt * 128:(rt + 1) * 128, :])
            for f in range(8):
                pt = ps1.tile([128, 128], F32, tag="tp")
                nc.tensor.transpose(pt[:], xrow[:, f * 128:(f + 1) * 128],
                                    ident[:])
                nc.scalar.copy(xqT[f][:, rt * 128:(rt + 1) * 128], pt[:])
                if xqr is not xqT:
                    nc.vector.tensor_copy(
                        xqr[f][:, rt * 128:(rt + 1) * 128], pt[:])

        # --- Q projection: qT[f] = (Wq.T @ xq.T + bq) * 0.125
        for f in range(8):
            wq = p1st.tile([128, 8, 128], MM, tag="wst")
            nc.sync.dma_start(
                wq[:], _r(wqkv_t[:, f, :, :].rearrange("k p n -> p k n")))
            pp = ps1p.tile([128, 2 * CH], F32, tag="proj")
            for kc in range(8):
                nc.tensor.matmul(pp[:], wq[:, kc, :], xqr[kc][:],
                                 start=(kc == 0), stop=(kc == 7))
            nc.scalar.activation(qT[f][:], pp[:], AF.Identity,
                                 bias=bias["bqs"][:, f:f + 1], scale=0.125)

        # --- per 512-row block: transpose xb, project k (spill) and v
        for rb in range(4):
            xbT = [p1x.tile([128, 4, 512], MM, tag="xbt", name=f"xbT{i}")
                   for i in range(2)]
            for rt in range(4):
                xrow = p1.tile([128, 1024], F32, tag="xrow")
                nc.sync.dma_start(
                    xrow[:],
                    xb[rb * 512 + rt * 128:rb * 512 + (rt + 1) * 128, :])
                for kc in range(8):
                    pt = ps1.tile([128, 128], F32, tag="tp")
                    nc.tensor.transpose(pt[:], xrow[:, kc * 128:(kc + 1) * 128],
                                        ident[:])
                    nc.vector.tensor_copy(
                        xbT[kc // 4][:, kc % 4, rt * 128:(rt + 1) * 128], pt[:])

            # kT: features f*128..+128, rows rb*512..+512 -> kt_dram
            for f in range(8):
                wk = p1st.tile([128, 8, 128], MM, tag="wst")
                nc.sync.dma_start(
                    wk[:], _r(wqkv_t[:, 8 + f, :, :].rearrange("k p n -> p k n")))
                pp = ps1p.tile([128, 512], F32, tag="proj")
                for kc in range(8):
                    nc.tensor.matmul(pp[:], wk[:, kc, :],
                                     xbT[kc // 4][:, kc % 4, :],
                                     start=(kc == 0), stop=(kc == 7))
                ks = p1.tile([128, 512], MM, tag="kstage")
                nc.scalar.activation(ks[:], pp[:], AF.Identity,
                                     bias=bias["bk"][:, f:f + 1])
                nc.gpsimd.dma_start(
                    kt_dram[f * 128:(f + 1) * 128, rb * 512:(rb + 1) * 512],
                    ks[:])

            # v: rows rb*512..+512, all 1024 v-cols -> v_aug tiles
            for nb in range(2):
                wv = [p1wv.tile([128, 512], MM, tag="wv", name=f"wv{i}")
                  for i in range(8)]
                for kc in range(8):
                    nc.sync.dma_start(
                        wv[kc][:],
                        _r(wqkv[kc * 128:(kc + 1) * 128,
                                2048 + nb * 512:2048 + (nb + 1) * 512]))
                for rt in range(4):
                    pp = ps1p.tile([128, 512], F32, tag="proj")
                    for kc in range(8):
                        nc.tensor.matmul(
                            pp[:],
                            xbT[kc // 4][:, kc % 4, rt * 128:(rt + 1) * 128],
                            wv[kc][:], start=(kc == 0), stop=(kc == 7))
                    nc.vector.tensor_copy(
                        v_aug[rb * 4 + rt][:, nb * 8:(nb + 1) * 8, 0:D],
                        pp[:].rearrange("p (h d) -> p h d", d=D))


def _phase2(nc, tc, AF, OP, t_, bias, ones, v_aug, qT, xqT, x2T):
    """Attention per head, transposed flow; writes x2T = xq + attn_out (fp32).

    kv rows arrive block-permuted (8 blocks of 256): slot 3 = the A-chunk's
    diagonal block, slot 7 = the B-chunk's. Prefix blocks need only a
    per-block additive gate (0 / -1e9), folded into the Exp activation's
    bias, so only diagonal blocks pay a DVE mask-add (constant tri mask).
    AV matmuls and the normalize/residual chain are software-pipelined one
    step behind so the in-order PE queue never stalls on DVE/ACT latency."""
    gates, maskd, kt_dram = t_["gates"], t_["maskd"], t_["kt_dram"]
    with tc.tile_pool(name="p2m", bufs=1) as p2m, \
         tc.tile_pool(name="p2k", bufs=2) as p2k, \
         tc.tile_pool(name="p2w", bufs=6) as p2w, \
         tc.tile_pool(name="ps2s", bufs=3, space="PSUM") as ps2s, \
         tc.tile_pool(name="ps2o", bufs=3, space="PSUM") as ps2o, \
         tc.tile_pool(name="ps2b", bufs=2, space="PSUM") as ps2b:

        # gates[:, 2*s + (0:A,1:B)] : bias column for slot s
        gt = p2m.tile([128, 16], F32, tag="gt")
        nc.sync.dma_start(gt[:], gates)
        md = p2m.tile([128, 2, CH], F32, tag="md")
        nc.sync.dma_start(md[:], maskd.rearrange("(c p) q -> p c q", p=128))

        # odd-head residual operands shifted down to partitions 0:64
        xq_lo = [p2m.tile([128, 2 * CH], F32, tag=f"xql{f}", name=f"xq_lo{f}")
                 for f in range(8)]
        for f in range(8):
            nc.gpsimd.dma_start(xq_lo[f][0:D, :], xqT[f][D:128, :])

        pending = []          # deferred one-step work (closures)

        def flush():
            for fn in pending:
                fn()
            pending.clear()

        for f in range(8):
            kth = p2k.tile([128, T], MM, tag="kth")
            nc.sync.dma_start(kth[:], kt_dram[f * 128:(f + 1) * 128, :])
            x2lo = p2w.tile([128, 2 * CH], F32, tag="x2lo", name=f"x2lo{f}",
                            bufs=2)
            for hp in range(2):
                h = 2 * f + hp
                po = 64 * hp
                qh = qT[f][po:po + D, :]
                bv_h = bias["bv"] if hp == 0 else bias["bvo"]
                for (qi, qoff, nblk) in ((0, 0, 4), (1, CH, 8)):
                    diag = nblk - 1
                    acc = ps2o.tile([128, CH], F32, tag="po")
                    for blk in range(nblk):
                        ps = ps2s.tile([128, 2, CH], F32, tag="ps")
                        for s in range(2):
                            c = 2 * blk + s
                            nc.tensor.matmul(
                                ps[:, s, :],
                                kth[po:po + D, c * 128:(c + 1) * 128],
                                qh[:, qoff:qoff + CH], start=True, stop=True)
                        ex = p2w.tile([128, 2, CH], MM, tag="ex", bufs=4)
                        if blk == diag:
                            sm = p2w.tile([128, 2, CH], F32, tag="sm", bufs=2)
                            nc.vector.tensor_tensor(out=sm[:], in0=ps[:],
                                                    in1=md[:], op=OP.add)
                            nc.scalar.activation(ex[:], sm[:], AF.Exp)
                        else:
                            nc.scalar.activation(
                                ex[:], ps[:], AF.Exp,
                                bias=gt[:, 2 * blk + qi:2 * blk + qi + 1])
                        flush()

                        def mk_avs(ex=ex, blk=blk, h=h, acc=acc, nblk=nblk):
                            def go():
                                for s in range(2):
                                    c = 2 * blk + s
                                    nc.tensor.matmul(
                                        acc[0:D + 1, :], v_aug[c][:, h, :],
                                        ex[:, s, :], start=(c == 0),
                                        stop=(c == 2 * nblk - 1))
                            return go
                        pending.append(mk_avs())

                    def mk_fin(acc=acc, hp=hp, qoff=qoff, f=f, bv_h=bv_h,
                               x2lo=x2lo):
                        def go():
                            rec = p2w.tile([128, CH], F32, tag="rec", bufs=1)
                            nc.vector.reciprocal(rec[D:D + 1, :],
                                                 acc[D:D + 1, :])
                            pb = ps2b.tile([128, CH], F32, tag="pb")
                            nc.tensor.matmul(pb[0:D, :], ones[D:D + 1, :],
                                             rec[D:D + 1, :], start=True,
                                             stop=True)
                            sb = p2w.tile([128, CH], F32, tag="sb", bufs=1)
                            nc.scalar.copy(sb[0:D, :], pb[0:D, :])
                            tt = p2w.tile([128, CH], F32, tag="tt", bufs=1)
                            nc.vector.tensor_tensor(
                                out=tt[0:D, :], in0=acc[0:D, :],
                                in1=sb[0:D, :], op=OP.mult)
                            nc.vector.tensor_scalar_add(
                                tt[0:D, :], tt[0:D, :], bv_h[0:D, f:f + 1])
                            if hp == 0:
                                nc.vector.tensor_tensor(
                                    out=x2T[f][0:D, qoff:qoff + CH],
                                    in0=tt[0:D, :],
                                    in1=xqT[f][0:D, qoff:qoff + CH],
                                    op=OP.add)
                            else:
                                nc.vector.tensor_tensor(
                                    out=x2lo[0:D, qoff:qoff + CH],
                                    in0=tt[0:D, :],
                                    in1=xq_lo[f][0:D, qoff:qoff + CH],
                                    op=OP.add)
                                if qoff == CH:
                                    nc.gpsimd.dma_start(x2T[f][D:128, :],
                                                        x2lo[0:D, :])
                        return go
                    pending.append(mk_fin())
        flush()


def _phase3(nc, tc, AF, OP, t_, bias, ident, x2T, out):
    """MLP (transposed) + residual + transpose back to natural layout."""
    w1_t, w2_t = t_["w1_t"], t_["w2_t"]
    with tc.tile_pool(name="p3h", bufs=8) as p3h, \
         tc.tile_pool(name="p3w1", bufs=4) as p3w1, \
         tc.tile_pool(name="p3w2", bufs=2) as p3w2, \
         tc.tile_pool(name="p3s", bufs=2) as p3s, \
         tc.tile_pool(name="p3y", bufs=1) as p3y, \
         tc.tile_pool(name="ps3p", bufs=4, space="PSUM") as ps3p, \
         tc.tile_pool(name="ps3t", bufs=2, space="PSUM") as ps3t:

        if USE_F32R:
            x2r = [p3y.tile([128, 2 * CH], F32R, tag=f"x2r{f}",
                           name=f"x2r{f}") for f in range(8)]
            for f in range(8):
                nc.vector.tensor_copy(x2r[f][:], x2T[f][:])
        else:
            x2r = x2T

        hT = [p3h.tile([128, 4, 2 * CH], MM, tag="ht", name=f"hT{i}")
              for i in range(8)]
        for m in range(32):
            w1s = p3w1.tile([128, 8, 128], MM, tag="w1s")
            nc.sync.dma_start(
                w1s[:], _r(w1_t[m, :, :, :].rearrange("k p n -> p k n")))
            pp = ps3p.tile([128, 2 * CH], F32, tag="proj")
            for kc in range(8):
                nc.tensor.matmul(pp[:], w1s[:, kc, :], x2r[kc][:],
                                 start=(kc == 0), stop=(kc == 7))
            nc.scalar.activation(hT[m // 4][:, m % 4, :], pp[:], AF.Relu,
                                 bias=bias["b1"][:, m:m + 1])

        yt = []
        for mo in range(8):
            w2s = p3w2.tile([128, 32, 128], MM, tag="w2s")
            nc.sync.dma_start(
                w2s[:], _r(w2_t[mo, :, :, :].rearrange("k p n -> p k n")))
            pp = ps3p.tile([128, 2 * CH], F32, tag="proj")
            for kc in range(32):
                nc.tensor.matmul(pp[:], w2s[:, kc, :], hT[kc // 4][:, kc % 4, :],
                                 start=(kc == 0), stop=(kc == 31))
            ys = p3y.tile([128, 2 * CH], F32, tag=f"yt{mo}", name=f"ys{mo}")
            nc.scalar.activation(ys[:], pp[:], AF.Identity,
                                 bias=bias["b2"][:, mo:mo + 1])
            nc.vector.tensor_tensor(out=ys[:], in0=ys[:], in1=x2T[mo][:],
                                    op=OP.add)
            yt.append(ys)

        # transpose back: out[rows, feats]
        for rt in range(4):
            onat = p3s.tile([128, 1024], F32, tag="onat")
            for mo in range(8):
                pt = ps3t.tile([128, 128], F32, tag="tp")
                nc.tensor.transpose(pt[:], yt[mo][:, rt * 128:(rt + 1) * 128],
                                    ident[:])
                nc.scalar.copy(onat[:, mo * 128:(mo + 1) * 128], pt[:])
            nc.gpsimd.dma_start(out[rt * 128:(rt + 1) * 128, :], onat[:])


# --------------------------------------------------------------- host driver
def _install_ntff_hook():
    """The container's antenv stub lacks axon_hooks; provide it so
    run_bass_kernel_spmd(trace=True) can capture NTFF profiles via libaxon."""
    import types

    try:
        import antenv.axon_hooks  # noqa: F401
        return
    except ImportError:
        pass
    holder = {"h": None}
    mod = types.ModuleType("antenv.axon_hooks")
    mod.set_axon_ntff_profile_hook = lambda h: holder.__setitem__("h", h)
    mod.get_axon_ntff_profile_hook = lambda: holder["h"]
    sys.modules["antenv.axon_hooks"] = mod
    import antenv

    antenv.axon_hooks = mod
    if "/root/.axon_site" not in sys.path:
        sys.path.insert(0, "/root/.axon_site")
    from trn_agent_boot.trn_boot import _ntff_profile_via_ctypes

    so = "/opt/axon/libaxon_pjrt.so"
    if os.path.exists(so):
        mod.set_axon_ntff_profile_hook(_ntff_profile_via_ctypes(so))


def _get_program():
    key = ("v1", USE_F32R)
    if key not in _prog_cache:
        nc = _build_program()
        _legalize_waits(nc)
        _prog_cache[key] = nc
    return _prog_cache[key]


def _prep_shared(Wqkv, W1, W2, bqkv, b1, b2):
    bv_ = bqkv[2 * N:]
    bvo = np.zeros(N, np.float32)
    bvo.reshape(8, 128)[:, 0:64] = bv_.reshape(8, 128)[:, 64:128]
    wqkv_t = np.ascontiguousarray(
        Wqkv.reshape(8, 128, 24, 128).transpose(0, 2, 1, 3))
    w1_t = np.ascontiguousarray(
        W1.reshape(8, 128, 32, 128).transpose(2, 0, 1, 3))
    w2_t = np.ascontiguousarray(
        W2.reshape(32, 128, 8, 128).transpose(2, 0, 1, 3))
    return {
        "wqkv_t": wqkv_t, "wqkv": np.ascontiguousarray(Wqkv),
        "w1_t": w1_t, "w2_t": w2_t,
        "bqs": np.ascontiguousarray(bqkv[:N] * 0.125),
        "bk": np.ascontiguousarray(bqkv[N:2 * N]),
        "bv": np.ascontiguousarray(bv_),
        "bvo": bvo,
        "b1": np.ascontiguousarray(b1), "b2": np.ascontiguousarray(b2),
    }


def _core_chunks(c):
    b, j = c // 4, c % 4
    return b, j, 7 - j


def _slot_blocks(j):
    # slot order of the 8 kv row-blocks: slot 3 = A diag (block j),
    # slot 7 = B diag (block 7-j), others ascending.
    other = [b for b in range(8) if b not in (j, 7 - j)]
    return [other[0], other[1], other[2], j, other[3], other[4], other[5],
            7 - j]


def _make_gates(j):
    slots = _slot_blocks(j)
    g = np.full((128, 16), -1e9, np.float32)
    for s in range(8):
        if s != 3 and slots[s] < j:
            g[:, 2 * s] = 0.0          # allowed for A
        if s != 7 and slots[s] < 7 - j:
            g[:, 2 * s + 1] = 0.0      # allowed for B
    return g


_MASKD = np.where(np.arange(256)[:, None] <= np.arange(CH)[None, :],
                  0.0, -1e9).astype(np.float32)


def kernel(x, Wqkv, bqkv, W1, b1, W2, b2, _trace=False):
    x = np.asarray(x, dtype=np.float32)
    shared = _prep_shared(np.asarray(Wqkv, np.float32),
                          np.asarray(W1, np.float32),
                          np.asarray(W2, np.float32),
                          np.asarray(bqkv, np.float32),
                          np.asarray(b1, np.float32),
                          np.asarray(b2, np.float32))
    in_maps = []
    for c in range(NCORES):
        b, j, jb = _core_chunks(c)
        xqc = np.concatenate(
            [x[b, j * CH:(j + 1) * CH], x[b, jb * CH:(jb + 1) * CH]], axis=0)
        xbp = x[b].reshape(8, CH, N)[_slot_blocks(j)].reshape(T, N)
        in_maps.append({
            **shared,
            "xb": np.ascontiguousarray(xbp),
            "xq": np.ascontiguousarray(xqc),
            "gates": _make_gates(j), "maskd": _MASKD,
        })

    nc = _get_program()
    if _trace:
        _install_ntff_hook()
    res = run_bass_kernel_spmd(nc, in_maps, list(range(NCORES)), trace=_trace)

    outf = np.empty((B, T, N), dtype=np.float32)
    for c in range(NCORES):
        b, j, jb = _core_chunks(c)
        o = res.results[c]["out"]
        outf[b, j * CH:(j + 1) * CH] = o[:CH]
        outf[b, jb * CH:(jb + 1) * CH] = o[CH:]
    if _trace:
        kernel.last_results = res
    return outf


# revision 17
# speedup vs baseline: 1.0258x; 1.0258x over previous
"""Trainium2 Bass kernel for a causal AttentionBlock (dense transformer).

Model (reference):
    qkv = x @ Wqkv + bqkv ; 16-head causal attention (no out-proj)
    x2  = x + attn_out
    out = x2 + relu(x2 @ W1 + b1) @ W2 + b2

x: [2, 2048, 1024] fp32. 8 NeuronCores.

Sharding (no collectives — on-chip collectives are too slow at these sizes):
data-parallel over (batch, query-chunk). Core c handles batch b = c//4 and the
balanced causal chunk pair (j, 7-j), j = c%4, of 8x256-row chunks, giving every
core the same 512 query rows. Each core redundantly projects K/V for its whole
batch (uniform SPMD program), computes attention for its rows with a shipped
additive causal mask, then the MLP for its rows. Host concatenates.

Everything on-chip runs transposed ([feature, row] layout) so that:
  - scoresT = kT.T @ qT needs no transposes of activations,
  - softmax denominators come free via a ones-column appended to V,
  - softmax max-subtraction is skipped (scores are bounded, exp is safe),
  - per-partition biases fold into PSUM-evacuation activations.
For odd heads the ones-column precedes V ([1|v] vs [v|1]) so the AV output
lands on partitions 64..127, aligned with the head's feature slice.
"""
import os
import sys

sys.path.insert(0, "/opt/trn_rl_repo")

import numpy as np

import bass_rust
import concourse.bass as bass
import concourse.mybir as mybir
import concourse.tile as tile
from concourse.bass_utils import run_bass_kernel_spmd

# ---------------------------------------------------------------- constants
B, T, N = 2, 2048, 1024
H, D = 16, 64
NCORES = 8
CH = 256               # query chunk rows
TKA, TKB = 1024, 2048  # uniform kv extents for chunk A / chunk B
F32 = mybir.dt.float32
F32R = mybir.dt.float32r

# Matmul input dtype: float32r (TF32-like, ~1e-3 max matmul rel err, 4x faster)
# or float32 (bit-accurate, 4 cycles/row). Flip with env KERNEL_F32R=0/1.
USE_F32R = os.environ.get("KERNEL_F32R", "1") == "1"
MM = F32R if USE_F32R else F32

_prog_cache = {}


def _r(ap):
    """View an fp32 DRAM AP as float32r for DMA into float32r tiles."""
    return ap.bitcast(F32R) if USE_F32R else ap


# ------------------------------------------------------------- wait legalizer
def _legalize_waits(nc):
    """This walrus build accepts <=1 sync wait on most instructions and 0 on
    fp32/fp32r Matmult (fused self-loading LDW). Move excess waits onto bare
    EventSemaphore instructions inserted before, on the same engine."""
    n_split = 0
    for fn in nc.m.functions:
        for blk in fn.blocks:
            insts = blk.instructions
            out = []
            for inst in insts:
                si = inst.sync_info
                waits = list(si.on_wait) if si is not None else []
                tname = type(inst).__name__
                if tname in ("InstMatmult", "InstMatmultMx"):
                    maxw = 0
                    for arg in inst.ins:
                        dt = getattr(arg, "dtype", None)
                        if dt is not None and mybir.dt.size(dt) == 2:
                            maxw = 1
                            break
                else:
                    maxw = 1
                if len(waits) > maxw:
                    extra = waits[:-maxw] if maxw else waits
                    keep = waits[-maxw:] if maxw else []
                    for k, w in enumerate(extra):
                        ev = mybir.InstEventSemaphore(
                            name=f"{inst.name}-lw{k}", ins=[], outs=[]
                        )
                        ev.engine = inst.engine
                        ev.sync_info = bass_rust.SyncInfo(on_wait=[w], on_update=[])
                        out.append(ev)
                        n_split += 1
                    inst.sync_info = bass_rust.SyncInfo(
                        on_wait=keep, on_update=list(si.on_update)
                    )
                out.append(inst)
            insts[:] = out
    return n_split


# ------------------------------------------------------------------- program
def _build_program():
    nc = bass.Bass("TRN2", debug=False, num_devices=NCORES)

    t_ = {}
    t_["xb"] = nc.dram_tensor("xb", [T, N], F32, kind="ExternalInput").ap()
    t_["xq"] = nc.dram_tensor("xq", [2 * CH, N], F32, kind="ExternalInput").ap()
    t_["wqkv_t"] = nc.dram_tensor("wqkv_t", [8, 24, 128, 128], F32,
                                  kind="ExternalInput").ap()
    t_["w1_t"] = nc.dram_tensor("w1_t", [32, 8, 128, 128], F32,
                                kind="ExternalInput").ap()
    t_["w2_t"] = nc.dram_tensor("w2_t", [8, 32, 128, 128], F32,
                                kind="ExternalInput").ap()
    for nm, sz in (("bqs", N), ("bk", N), ("bv", N), ("bvo", N),
                   ("b1", 4 * N), ("b2", N)):
        t_[nm] = nc.dram_tensor(nm, [sz], F32, kind="ExternalInput").ap()
    t_["gates"] = nc.dram_tensor("gates", [128, 16], F32,
                                 kind="ExternalInput").ap()
    t_["maskd"] = nc.dram_tensor("maskd", [256, CH], F32,
                                 kind="ExternalInput").ap()
    t_["out"] = nc.dram_tensor("out", [2 * CH, N], F32, kind="ExternalOutput").ap()
    t_["kt_dram"] = nc.dram_tensor("kt_scratch", [N, T], MM).ap()

    with tile.TileContext(nc) as tc:
        _emit(nc, tc, t_)
    return nc


def _emit(nc, tc, t_):
    AF = mybir.ActivationFunctionType
    OP = mybir.AluOpType

    with tc.tile_pool(name="const", bufs=1) as const:
        ident = const.tile([128, 128], F32)
        nc.gpsimd.memset(ident[:], 0.0)
        nc.gpsimd.affine_select(
            out=ident[:], in_=ident[:], compare_op=OP.not_equal, fill=1.0,
            base=0, pattern=[[-1, 128]], channel_multiplier=1,
        )
        ones = const.tile([128, 64], F32)
        nc.vector.memset(ones[:], 1.0)
        bias = {}
        for nm, w in (("bqs", 8), ("bk", 8), ("bv", 8), ("bvo", 8),
                      ("b1", 32), ("b2", 8)):
            bias[nm] = const.tile([128, w], F32, name=f"b_{nm}")
            nc.sync.dma_start(bias[nm][:], t_[nm].rearrange("(f p) -> p f", p=128))

        with tc.tile_pool(name="x2t", bufs=8) as px2t:
            x2T = [px2t.tile([128, 2 * CH], F32, tag="x2t", name=f"x2T{i}")
                   for i in range(8)]

            with tc.tile_pool(name="keep", bufs=1) as keep:
                # v_aug[rt]: [128 kv-rows, head h -> [v|1] (even) / [1|v] (odd)]
                v_aug = [keep.tile([128, H, D + 1], MM, tag=f"va{rt}",
                                   name=f"va{rt}") for rt in range(T // 128)]
                qT = [keep.tile([128, 2 * CH], MM, tag=f"qt{f}",
                              name=f"qT{f}") for f in range(8)]
                xqT = [keep.tile([128, 2 * CH], F32, tag=f"xqt{f}",
                               name=f"xqT{f}") for f in range(8)]
                if USE_F32R:
                    xqr = [keep.tile([128, 2 * CH], F32R, tag=f"xqr{f}",
                                   name=f"xqr{f}") for f in range(8)]
                else:
                    xqr = xqT

                _phase1(nc, tc, AF, OP, t_, bias, ident, v_aug, qT, xqT, xqr)
                _phase2(nc, tc, AF, OP, t_, bias, ones, v_aug, qT, xqT, x2T)
            _phase3(nc, tc, AF, OP, t_, bias, ident, x2T, t_["out"])


def _phase1(nc, tc, AF, OP, t_, bias, ident, v_aug, qT, xqT, xqr):
    """Transposes + Q/K/V projections. kT spills to DRAM; v_aug stays in SBUF."""
    xb, xq, wqkv_t, kt_dram = (t_["xb"], t_["xq"], t_["wqkv_t"],
                               t_["kt_dram"])
    with tc.tile_pool(name="p1", bufs=2) as p1, \
         tc.tile_pool(name="p1st", bufs=4) as p1st, \
         tc.tile_pool(name="p1wv", bufs=9) as p1wv, \
         tc.tile_pool(name="p1x", bufs=2) as p1x, \
         tc.tile_pool(name="ps1", bufs=2, space="PSUM") as ps1, \
         tc.tile_pool(name="ps1p", bufs=4, space="PSUM") as ps1p:

        # ones column of v_aug (all heads: [v | 1]); memset lacks an fp32r
        # encoding, so write the bits through a uint32 view
        for rt in range(T // 128):
            if MM == F32:
                nc.vector.memset(v_aug[rt][:, :, D:D + 1], 1.0)
            else:
                nc.vector.memset(
                    v_aug[rt][:, :, D:D + 1].bitcast(mybir.dt.uint32),
                    0x3F800000)

        # --- xq transpose: xqT[f] [128, 512] (+ fp32r copy for matmul use)
        for rt in range(4):
            xrow = p1.tile([128, 1024], F32, tag="xrow")
            nc.sync.dma_start(xrow[:], xq[rt * 128:(rt + 1) * 128, :])
            for f in range(8):
                pt = ps1.tile([128, 128], F32, tag="tp")
                nc.tensor.transpose(pt[:], xrow[:, f * 128:(f + 1) * 128],
                                    ident[:])
                nc.scalar.copy(xqT[f][:, rt * 128:(rt + 1) * 128], pt[:])
                if xqr is not xqT:
                    nc.vector.tensor_copy(
                        xqr[f][:, rt * 128:(rt + 1) * 128], pt[:])

        # --- Q projection: qT[f] = (Wq.T @ xq.T + bq) * 0.125
        for f in range(8):
            wq = p1st.tile([128, 8, 128], MM, tag="wst")
            nc.sync.dma_start(
                wq[:], _r(wqkv_t[:, f, :, :].rearrange("k p n -> p k n")))
            pp = ps1p.tile([128, 2 * CH], F32, tag="proj")
            for kc in range(8):
                nc.tensor.matmul(pp[:], wq[:, kc, :], xqr[kc][:],
                                 start=(kc == 0), stop=(kc == 7))
            nc.scalar.activation(qT[f][:], pp[:], AF.Identity,
                                 bias=bias["bqs"][:, f:f + 1], scale=0.125)

        # --- per 512-row block: transpose xb, project k (spill) and v
        for rb in range(4):
            xbT = [p1x.tile([128, 4, 512], MM, tag="xbt", name=f"xbT{i}")
                   for i in range(2)]
            for rt in range(4):
                xrow = p1.tile([128, 1024], F32, tag="xrow")
                nc.sync.dma_start(
                    xrow[:],
                    xb[rb * 512 + rt * 128:rb * 512 + (rt + 1) * 128, :])
                for kc in range(8):
                    pt = ps1.tile([128, 128], F32, tag="tp")
                    nc.tensor.transpose(pt[:], xrow[:, kc * 128:(kc + 1) * 128],
                                        ident[:])
                    nc.vector.tensor_copy(
                        xbT[kc // 4][:, kc % 4, rt * 128:(rt + 1) * 128], pt[:])

            # kT: features f*128..+128, rows rb*512..+512 -> kt_dram
            for f in range(8):
                wk = p1st.tile([128, 8, 128], MM, tag="wst")
                nc.sync.dma_start(
                    wk[:], _r(wqkv_t[:, 8 + f, :, :].rearrange("k p n -> p k n")))
                pp = ps1p.tile([128, 512], F32, tag="proj")
                for kc in range(8):
                    nc.tensor.matmul(pp[:], wk[:, kc, :],
                                     xbT[kc // 4][:, kc % 4, :],
                                     start=(kc == 0), stop=(kc == 7))
                ks = p1.tile([128, 512], MM, tag="kstage")
                nc.scalar.activation(ks[:], pp[:], AF.Identity,
                                     bias=bias["bk"][:, f:f + 1])
                nc.gpsimd.dma_start(
                    kt_dram[f * 128:(f + 1) * 128, rb * 512:(rb + 1) * 512],
                    ks[:])

            # v: rows rb*512..+512, all 1024 v-cols -> v_aug tiles
            for nb in range(2):
                wv = [p1wv.tile([128, 512], MM, tag="wv", name=f"wv{i}")
                  for i in range(8)]
                for kc in range(8):
                    nc.sync.dma_start(
                        wv[kc][:].rearrange("p (m n) -> p m n", m=4),
                        _r(wqkv_t[kc, 16 + nb * 4:16 + (nb + 1) * 4, :, :]
                           .rearrange("m p n -> p m n")))
                for rt in range(4):
                    pp = ps1p.tile([128, 512], F32, tag="proj")
                    for kc in range(8):
                        nc.tensor.matmul(
                            pp[:],
                            xbT[kc // 4][:, kc % 4, rt * 128:(rt + 1) * 128],
                            wv[kc][:], start=(kc == 0), stop=(kc == 7))
                    nc.vector.tensor_copy(
                        v_aug[rb * 4 + rt][:, nb * 8:(nb + 1) * 8, 0:D],
                        pp[:].rearrange("p (h d) -> p h d", d=D))


def _phase2(nc, tc, AF, OP, t_, bias, ones, v_aug, qT, xqT, x2T):
    """Attention per head, transposed flow; writes x2T = xq + attn_out (fp32).

    kv rows arrive block-permuted (8 blocks of 256): slot 3 = the A-chunk's
    diagonal block, slot 7 = the B-chunk's. Prefix blocks need only a
    per-block additive gate (0 / -1e9), folded into the Exp activation's
    bias, so only diagonal blocks pay a DVE mask-add (constant tri mask).
    AV matmuls and the normalize/residual chain are software-pipelined one
    step behind so the in-order PE queue never stalls on DVE/ACT latency."""
    gates, maskd, kt_dram = t_["gates"], t_["maskd"], t_["kt_dram"]
    with tc.tile_pool(name="p2m", bufs=1) as p2m, \
         tc.tile_pool(name="p2k", bufs=2) as p2k, \
         tc.tile_pool(name="p2w", bufs=6) as p2w, \
         tc.tile_pool(name="ps2s", bufs=3, space="PSUM") as ps2s, \
         tc.tile_pool(name="ps2o", bufs=3, space="PSUM") as ps2o, \
         tc.tile_pool(name="ps2b", bufs=2, space="PSUM") as ps2b:

        # gates[:, 2*s + (0:A,1:B)] : bias column for slot s
        gt = p2m.tile([128, 16], F32, tag="gt")
        nc.sync.dma_start(gt[:], gates)
        md = p2m.tile([128, 2, CH], F32, tag="md")
        nc.sync.dma_start(md[:], maskd.rearrange("(c p) q -> p c q", p=128))

        # odd-head residual operands shifted down to partitions 0:64
        xq_lo = [p2m.tile([128, 2 * CH], F32, tag=f"xql{f}", name=f"xq_lo{f}")
                 for f in range(8)]
        for f in range(8):
            nc.gpsimd.dma_start(xq_lo[f][0:D, :], xqT[f][D:128, :])

        pending = []          # deferred one-step work (closures)

        def flush():
            for fn in pending:
                fn()
            pending.clear()

        for f in range(8):
            kth = p2k.tile([128, T], MM, tag="kth")
            nc.sync.dma_start(kth[:], kt_dram[f * 128:(f + 1) * 128, :])
            x2lo = p2w.tile([128, 2 * CH], F32, tag="x2lo", name=f"x2lo{f}",
                            bufs=2)
            for hp in range(2):
                h = 2 * f + hp
                po = 64 * hp
                qh = qT[f][po:po + D, :]
                bv_h = bias["bv"] if hp == 0 else bias["bvo"]
                for (qi, qoff, nblk) in ((0, 0, 4), (1, CH, 8)):
                    diag = nblk - 1
                    acc = ps2o.tile([128, CH], F32, tag="po")
                    for blk in range(nblk):
                        ps = ps2s.tile([128, 2, CH], F32, tag="ps")
                        for s in range(2):
                            c = 2 * blk + s
                            nc.tensor.matmul(
                                ps[:, s, :],
                                kth[po:po + D, c * 128:(c + 1) * 128],
                                qh[:, qoff:qoff + CH], start=True, stop=True)
                        ex = p2w.tile([128, 2, CH], MM, tag="ex", bufs=4)
                        if blk == diag:
                            sm = p2w.tile([128, 2, CH], F32, tag="sm", bufs=2)
                            nc.vector.tensor_tensor(out=sm[:], in0=ps[:],
                                                    in1=md[:], op=OP.add)
                            nc.scalar.activation(ex[:], sm[:], AF.Exp)
                        else:
                            nc.scalar.activation(
                                ex[:], ps[:], AF.Exp,
                                bias=gt[:, 2 * blk + qi:2 * blk + qi + 1])
                        flush()

                        def mk_avs(ex=ex, blk=blk, h=h, acc=acc, nblk=nblk):
                            def go():
                                for s in range(2):
                                    c = 2 * blk + s
                                    nc.tensor.matmul(
                                        acc[0:D + 1, :], v_aug[c][:, h, :],
                                        ex[:, s, :], start=(c == 0),
                                        stop=(c == 2 * nblk - 1))
                            return go
                        pending.append(mk_avs())

                    def mk_fin(acc=acc, hp=hp, qoff=qoff, f=f, bv_h=bv_h,
                               x2lo=x2lo):
                        def go():
                            rec = p2w.tile([128, CH], F32, tag="rec", bufs=1)
                            nc.vector.reciprocal(rec[D:D + 1, :],
                                                 acc[D:D + 1, :])
                            pb = ps2b.tile([128, CH], F32, tag="pb")
                            nc.tensor.matmul(pb[0:D, :], ones[D:D + 1, :],
                                             rec[D:D + 1, :], start=True,
                                             stop=True)
                            sb = p2w.tile([128, CH], F32, tag="sb", bufs=1)
                            nc.scalar.copy(sb[0:D, :], pb[0:D, :])
                            tt = p2w.tile([128, CH], F32, tag="tt", bufs=1)
                            nc.vector.tensor_tensor(
                                out=tt[0:D, :], in0=acc[0:D, :],
                                in1=sb[0:D, :], op=OP.mult)
                            nc.vector.tensor_scalar_add(
                                tt[0:D, :], tt[0:D, :], bv_h[0:D, f:f + 1])
                            if hp == 0:
                                nc.vector.tensor_tensor(
                                    out=x2T[f][0:D, qoff:qoff + CH],
                                    in0=tt[0:D, :],
                                    in1=xqT[f][0:D, qoff:qoff + CH],
                                    op=OP.add)
                            else:
                                nc.vector.tensor_tensor(
                                    out=x2lo[0:D, qoff:qoff + CH],
                                    in0=tt[0:D, :],
                                    in1=xq_lo[f][0:D, qoff:qoff + CH],
                                    op=OP.add)
                                if qoff == CH:
                                    nc.gpsimd.dma_start(x2T[f][D:128, :],
                                                        x2lo[0:D, :])
                        return go
                    pending.append(mk_fin())
        flush()


def _phase3(nc, tc, AF, OP, t_, bias, ident, x2T, out):
    """MLP (transposed) + residual + transpose back to natural layout."""
    w1_t, w2_t = t_["w1_t"], t_["w2_t"]
    with tc.tile_pool(name="p3h", bufs=8) as p3h, \
         tc.tile_pool(name="p3w1", bufs=4) as p3w1, \
         tc.tile_pool(name="p3w2", bufs=2) as p3w2, \
         tc.tile_pool(name="p3s", bufs=2) as p3s, \
         tc.tile_pool(name="p3y", bufs=1) as p3y, \
         tc.tile_pool(name="ps3p", bufs=4, space="PSUM") as ps3p, \
         tc.tile_pool(name="ps3t", bufs=2, space="PSUM") as ps3t:

        if USE_F32R:
            x2r = [p3y.tile([128, 2 * CH], F32R, tag=f"x2r{f}",
                           name=f"x2r{f}") for f in range(8)]
            for f in range(8):
                nc.vector.tensor_copy(x2r[f][:], x2T[f][:])
        else:
            x2r = x2T

        hT = [p3h.tile([128, 4, 2 * CH], MM, tag="ht", name=f"hT{i}")
              for i in range(8)]
        for m in range(32):
            w1s = p3w1.tile([128, 8, 128], MM, tag="w1s")
            nc.sync.dma_start(
                w1s[:], _r(w1_t[m, :, :, :].rearrange("k p n -> p k n")))
            pp = ps3p.tile([128, 2 * CH], F32, tag="proj")
            for kc in range(8):
                nc.tensor.matmul(pp[:], w1s[:, kc, :], x2r[kc][:],
                                 start=(kc == 0), stop=(kc == 7))
            nc.scalar.activation(hT[m // 4][:, m % 4, :], pp[:], AF.Relu,
                                 bias=bias["b1"][:, m:m + 1])

        yt = []
        for mo in range(8):
            w2s = p3w2.tile([128, 32, 128], MM, tag="w2s")
            nc.sync.dma_start(
                w2s[:], _r(w2_t[mo, :, :, :].rearrange("k p n -> p k n")))
            pp = ps3p.tile([128, 2 * CH], F32, tag="proj")
            for kc in range(32):
                nc.tensor.matmul(pp[:], w2s[:, kc, :], hT[kc // 4][:, kc % 4, :],
                                 start=(kc == 0), stop=(kc == 31))
            ys = p3y.tile([128, 2 * CH], F32, tag=f"yt{mo}", name=f"ys{mo}")
            nc.scalar.activation(ys[:], pp[:], AF.Identity,
                                 bias=bias["b2"][:, mo:mo + 1])
            nc.vector.tensor_tensor(out=ys[:], in0=ys[:], in1=x2T[mo][:],
                                    op=OP.add)
            yt.append(ys)

        # transpose back: out[rows, feats]
        for rt in range(4):
            onat = p3s.tile([128, 1024], F32, tag="onat")
            for mo in range(8):
                pt = ps3t.tile([128, 128], F32, tag="tp")
                nc.tensor.transpose(pt[:], yt[mo][:, rt * 128:(rt + 1) * 128],
                                    ident[:])
                nc.scalar.copy(onat[:, mo * 128:(mo + 1) * 128], pt[:])
            nc.gpsimd.dma_start(out[rt * 128:(rt + 1) * 128, :], onat[:])


# --------------------------------------------------------------- host driver
def _install_ntff_hook():
    """The container's antenv stub lacks axon_hooks; provide it so
    run_bass_kernel_spmd(trace=True) can capture NTFF profiles via libaxon."""
    import types

    try:
        import antenv.axon_hooks  # noqa: F401
        return
    except ImportError:
        pass
    holder = {"h": None}
    mod = types.ModuleType("antenv.axon_hooks")
    mod.set_axon_ntff_profile_hook = lambda h: holder.__setitem__("h", h)
    mod.get_axon_ntff_profile_hook = lambda: holder["h"]
    sys.modules["antenv.axon_hooks"] = mod
    import antenv

    antenv.axon_hooks = mod
    if "/root/.axon_site" not in sys.path:
        sys.path.insert(0, "/root/.axon_site")
    from trn_agent_boot.trn_boot import _ntff_profile_via_ctypes

    so = "/opt/axon/libaxon_pjrt.so"
    if os.path.exists(so):
        mod.set_axon_ntff_profile_hook(_ntff_profile_via_ctypes(so))


def _get_program():
    key = ("v1", USE_F32R)
    if key not in _prog_cache:
        nc = _build_program()
        _legalize_waits(nc)
        _prog_cache[key] = nc
    return _prog_cache[key]


def _prep_shared(Wqkv, W1, W2, bqkv, b1, b2):
    bv_ = bqkv[2 * N:]
    bvo = np.zeros(N, np.float32)
    bvo.reshape(8, 128)[:, 0:64] = bv_.reshape(8, 128)[:, 64:128]
    wqkv_t = np.ascontiguousarray(
        Wqkv.reshape(8, 128, 24, 128).transpose(0, 2, 1, 3))
    w1_t = np.ascontiguousarray(
        W1.reshape(8, 128, 32, 128).transpose(2, 0, 1, 3))
    w2_t = np.ascontiguousarray(
        W2.reshape(32, 128, 8, 128).transpose(2, 0, 1, 3))
    return {
        "wqkv_t": wqkv_t,
        "w1_t": w1_t, "w2_t": w2_t,
        "bqs": np.ascontiguousarray(bqkv[:N] * 0.125),
        "bk": np.ascontiguousarray(bqkv[N:2 * N]),
        "bv": np.ascontiguousarray(bv_),
        "bvo": bvo,
        "b1": np.ascontiguousarray(b1), "b2": np.ascontiguousarray(b2),
    }


def _core_chunks(c):
    b, j = c // 4, c % 4
    return b, j, 7 - j


def _slot_blocks(j):
    # slot order of the 8 kv row-blocks: slot 3 = A diag (block j),
    # slot 7 = B diag (block 7-j), others ascending.
    other = [b for b in range(8) if b not in (j, 7 - j)]
    return [other[0], other[1], other[2], j, other[3], other[4], other[5],
            7 - j]


def _make_gates(j):
    slots = _slot_blocks(j)
    g = np.full((128, 16), -1e9, np.float32)
    for s in range(8):
        if s != 3 and slots[s] < j:
            g[:, 2 * s] = 0.0          # allowed for A
        if s != 7 and slots[s] < 7 - j:
            g[:, 2 * s + 1] = 0.0      # allowed for B
    return g


_MASKD = np.where(np.arange(256)[:, None] <= np.arange(CH)[None, :],
                  0.0, -1e9).astype(np.float32)


def kernel(x, Wqkv, bqkv, W1, b1, W2, b2, _trace=False):
    x = np.asarray(x, dtype=np.float32)
    shared = _prep_shared(np.asarray(Wqkv, np.float32),
                          np.asarray(W1, np.float32),
                          np.asarray(W2, np.float32),
                          np.asarray(bqkv, np.float32),
                          np.asarray(b1, np.float32),
                          np.asarray(b2, np.float32))
    in_maps = []
    for c in range(NCORES):
        b, j, jb = _core_chunks(c)
        xqc = np.concatenate(
            [x[b, j * CH:(j + 1) * CH], x[b, jb * CH:(jb + 1) * CH]], axis=0)
        xbp = x[b].reshape(8, CH, N)[_slot_blocks(j)].reshape(T, N)
        in_maps.append({
            **shared,
            "xb": np.ascontiguousarray(xbp),
            "xq": np.ascontiguousarray(xqc),
            "gates": _make_gates(j), "maskd": _MASKD,
        })

    nc = _get_program()
    if _trace:
        _install_ntff_hook()
    res = run_bass_kernel_spmd(nc, in_maps, list(range(NCORES)), trace=_trace)

    outf = np.empty((B, T, N), dtype=np.float32)
    for c in range(NCORES):
        b, j, jb = _core_chunks(c)
        o = res.results[c]["out"]
        outf[b, j * CH:(j + 1) * CH] = o[:CH]
        outf[b, jb * CH:(jb + 1) * CH] = o[CH:]
    if _trace:
        kernel.last_results = res
    return outf


# revision 19
# speedup vs baseline: 1.0494x; 1.0230x over previous
"""Trainium2 Bass kernel for a causal AttentionBlock (dense transformer).

Model (reference):
    qkv = x @ Wqkv + bqkv ; 16-head causal attention (no out-proj)
    x2  = x + attn_out
    out = x2 + relu(x2 @ W1 + b1) @ W2 + b2

x: [2, 2048, 1024] fp32. 8 NeuronCores.

Sharding (no collectives — on-chip collectives are too slow at these sizes):
data-parallel over (batch, query-chunk). Core c handles batch b = c//4 and the
balanced causal chunk pair (j, 7-j), j = c%4, of 8x256-row chunks, giving every
core the same 512 query rows. Each core redundantly projects K/V for its whole
batch (uniform SPMD program), computes attention for its rows with a shipped
additive causal mask, then the MLP for its rows. Host concatenates.

Everything on-chip runs transposed ([feature, row] layout) so that:
  - scoresT = kT.T @ qT needs no transposes of activations,
  - softmax denominators come free via a ones-column appended to V,
  - softmax max-subtraction is skipped (scores are bounded, exp is safe),
  - per-partition biases fold into PSUM-evacuation activations.
For odd heads the ones-column precedes V ([1|v] vs [v|1]) so the AV output
lands on partitions 64..127, aligned with the head's feature slice.
"""
import os
import sys

sys.path.insert(0, "/opt/trn_rl_repo")

import numpy as np

import bass_rust
import concourse.bass as bass
import concourse.mybir as mybir
import concourse.tile as tile
from concourse.bass_utils import run_bass_kernel_spmd

# ---------------------------------------------------------------- constants
B, T, N = 2, 2048, 1024
H, D = 16, 64
NCORES = 8
CH = 256               # query chunk rows
TKA, TKB = 1024, 2048  # uniform kv extents for chunk A / chunk B
F32 = mybir.dt.float32
F32R = mybir.dt.float32r

# Matmul input dtype: float32r (TF32-like, ~1e-3 max matmul rel err, 4x faster)
# or float32 (bit-accurate, 4 cycles/row). Flip with env KERNEL_F32R=0/1.
USE_F32R = os.environ.get("KERNEL_F32R", "1") == "1"
MM = F32R if USE_F32R else F32

_prog_cache = {}


def _r(ap):
    """View an fp32 DRAM AP as float32r for DMA into float32r tiles."""
    return ap.bitcast(F32R) if USE_F32R else ap


# ------------------------------------------------------------- wait legalizer
def _legalize_waits(nc):
    """This walrus build accepts <=1 sync wait on most instructions and 0 on
    fp32/fp32r Matmult (fused self-loading LDW). Move excess waits onto bare
    EventSemaphore instructions inserted before, on the same engine."""
    n_split = 0
    for fn in nc.m.functions:
        for blk in fn.blocks:
            insts = blk.instructions
            out = []
            for inst in insts:
                si = inst.sync_info
                waits = list(si.on_wait) if si is not None else []
                tname = type(inst).__name__
                if tname in ("InstMatmult", "InstMatmultMx"):
                    maxw = 0
                    for arg in inst.ins:
                        dt = getattr(arg, "dtype", None)
                        if dt is not None and mybir.dt.size(dt) == 2:
                            maxw = 1
                            break
                else:
                    maxw = 1
                if len(waits) > maxw:
                    extra = waits[:-maxw] if maxw else waits
                    keep = waits[-maxw:] if maxw else []
                    for k, w in enumerate(extra):
                        ev = mybir.InstEventSemaphore(
                            name=f"{inst.name}-lw{k}", ins=[], outs=[]
                        )
                        ev.engine = inst.engine
                        ev.sync_info = bass_rust.SyncInfo(on_wait=[w], on_update=[])
                        out.append(ev)
                        n_split += 1
                    inst.sync_info = bass_rust.SyncInfo(
                        on_wait=keep, on_update=list(si.on_update)
                    )
                out.append(inst)
            insts[:] = out
    return n_split


# ------------------------------------------------------------------- program
def _build_program():
    nc = bass.Bass("TRN2", debug=False, num_devices=NCORES)

    t_ = {}
    t_["xb"] = nc.dram_tensor("xb", [T, N], F32, kind="ExternalInput").ap()
    t_["xq"] = nc.dram_tensor("xq", [2 * CH, N], F32, kind="ExternalInput").ap()
    t_["wqkv_t"] = nc.dram_tensor("wqkv_t", [8, 24, 128, 128], F32,
                                  kind="ExternalInput").ap()
    t_["w1_t"] = nc.dram_tensor("w1_t", [32, 8, 128, 128], F32,
                                kind="ExternalInput").ap()
    t_["w2_t"] = nc.dram_tensor("w2_t", [8, 32, 128, 128], F32,
                                kind="ExternalInput").ap()
    for nm, sz in (("bqs", N), ("bk", N), ("bv", N), ("bvo", N),
                   ("b1", 4 * N), ("b2", N)):
        t_[nm] = nc.dram_tensor(nm, [sz], F32, kind="ExternalInput").ap()
    t_["gates"] = nc.dram_tensor("gates", [128, 16], F32,
                                 kind="ExternalInput").ap()
    t_["maskd"] = nc.dram_tensor("maskd", [256, CH], F32,
                                 kind="ExternalInput").ap()
    t_["out"] = nc.dram_tensor("out", [2 * CH, N], F32, kind="ExternalOutput").ap()
    t_["kt_dram"] = nc.dram_tensor("kt_scratch", [N, T], MM).ap()

    with tile.TileContext(nc) as tc:
        _emit(nc, tc, t_)
    return nc


def _emit(nc, tc, t_):
    AF = mybir.ActivationFunctionType
    OP = mybir.AluOpType

    with tc.tile_pool(name="const", bufs=1) as const:
        ident = const.tile([128, 128], F32)
        nc.gpsimd.memset(ident[:], 0.0)
        nc.gpsimd.affine_select(
            out=ident[:], in_=ident[:], compare_op=OP.not_equal, fill=1.0,
            base=0, pattern=[[-1, 128]], channel_multiplier=1,
        )
        ones = const.tile([128, 64], F32)
        nc.vector.memset(ones[:], 1.0)
        bias = {}
        for nm, w in (("bqs", 8), ("bk", 8), ("bv", 8), ("bvo", 8),
                      ("b1", 32), ("b2", 8)):
            bias[nm] = const.tile([128, w], F32, name=f"b_{nm}")
            nc.sync.dma_start(bias[nm][:], t_[nm].rearrange("(f p) -> p f", p=128))

        with tc.tile_pool(name="x2t", bufs=8) as px2t:
            x2T = [px2t.tile([128, 2 * CH], F32, tag="x2t", name=f"x2T{i}")
                   for i in range(8)]

            with tc.tile_pool(name="keep", bufs=1) as keep:
                # v_aug[rt]: [128 kv-rows, head h -> [v|1] (even) / [1|v] (odd)]
                v_aug = [keep.tile([128, H, D + 1], MM, tag=f"va{rt}",
                                   name=f"va{rt}") for rt in range(T // 128)]
                qT = [keep.tile([128, 2 * CH], MM, tag=f"qt{f}",
                              name=f"qT{f}") for f in range(8)]
                xqT = [keep.tile([128, 2 * CH], F32, tag=f"xqt{f}",
                               name=f"xqT{f}") for f in range(8)]
                if USE_F32R:
                    xqr = [keep.tile([128, 2 * CH], F32R, tag=f"xqr{f}",
                                   name=f"xqr{f}") for f in range(8)]
                else:
                    xqr = xqT

                _phase1(nc, tc, AF, OP, t_, bias, ident, v_aug, qT, xqT, xqr)
                _phase2(nc, tc, AF, OP, t_, bias, ones, v_aug, qT, xqT, x2T)
            _phase3(nc, tc, AF, OP, t_, bias, ident, x2T, t_["out"])


def _phase1(nc, tc, AF, OP, t_, bias, ident, v_aug, qT, xqT, xqr):
    """Transposes + Q/K/V projections. kT spills to DRAM; v_aug stays in SBUF."""
    xb, xq, wqkv_t, kt_dram = (t_["xb"], t_["xq"], t_["wqkv_t"],
                               t_["kt_dram"])
    with tc.tile_pool(name="p1", bufs=2) as p1, \
         tc.tile_pool(name="p1st", bufs=4) as p1st, \
         tc.tile_pool(name="p1wv", bufs=9) as p1wv, \
         tc.tile_pool(name="p1x", bufs=2) as p1x, \
         tc.tile_pool(name="ps1", bufs=2, space="PSUM") as ps1, \
         tc.tile_pool(name="ps1p", bufs=6, space="PSUM") as ps1p:

        # ones column of v_aug (all heads: [v | 1]); memset lacks an fp32r
        # encoding, so write the bits through a uint32 view
        for rt in range(T // 128):
            if MM == F32:
                nc.vector.memset(v_aug[rt][:, :, D:D + 1], 1.0)
            else:
                nc.vector.memset(
                    v_aug[rt][:, :, D:D + 1].bitcast(mybir.dt.uint32),
                    0x3F800000)

        # --- xq transpose: xqT[f] [128, 512] (+ fp32r copy for matmul use)
        for rt in range(4):
            xrow = p1.tile([128, 1024], F32, tag="xrow")
            nc.sync.dma_start(xrow[:], xq[rt * 128:(rt + 1) * 128, :])
            for f in range(8):
                pt = ps1.tile([128, 128], F32, tag="tp")
                nc.tensor.transpose(pt[:], xrow[:, f * 128:(f + 1) * 128],
                                    ident[:])
                nc.scalar.copy(xqT[f][:, rt * 128:(rt + 1) * 128], pt[:])
                if xqr is not xqT:
                    nc.vector.tensor_copy(
                        xqr[f][:, rt * 128:(rt + 1) * 128], pt[:])

        # --- Q projection: qT[f] = (Wq.T @ xq.T + bq) * 0.125
        for f in range(8):
            wq = p1st.tile([128, 8, 128], MM, tag="wst")
            nc.sync.dma_start(
                wq[:], _r(wqkv_t[:, f, :, :].rearrange("k p n -> p k n")))
            pp = ps1p.tile([128, 2 * CH], F32, tag="proj")
            for kc in range(8):
                nc.tensor.matmul(pp[:], wq[:, kc, :], xqr[kc][:],
                                 start=(kc == 0), stop=(kc == 7))
            nc.scalar.activation(qT[f][:], pp[:], AF.Identity,
                                 bias=bias["bqs"][:, f:f + 1], scale=0.125)

        # --- per 512-row block: transpose xb, project k (spill) and v
        for rb in range(4):
            xbT = [p1x.tile([128, 4, 512], MM, tag="xbt", name=f"xbT{i}")
                   for i in range(2)]
            for rt in range(4):
                xrow = p1.tile([128, 1024], F32, tag="xrow")
                nc.sync.dma_start(
                    xrow[:],
                    xb[rb * 512 + rt * 128:rb * 512 + (rt + 1) * 128, :])
                for kc in range(8):
                    pt = ps1.tile([128, 128], F32, tag="tp")
                    nc.tensor.transpose(pt[:], xrow[:, kc * 128:(kc + 1) * 128],
                                        ident[:])
                    nc.vector.tensor_copy(
                        xbT[kc // 4][:, kc % 4, rt * 128:(rt + 1) * 128], pt[:])

            # kT: features f*128..+128, rows rb*512..+512 -> kt_dram
            for f in range(8):
                wk = p1st.tile([128, 8, 128], MM, tag="wst")
                nc.sync.dma_start(
                    wk[:], _r(wqkv_t[:, 8 + f, :, :].rearrange("k p n -> p k n")))
                pp = ps1p.tile([128, 512], F32, tag="proj")
                for kc in range(8):
                    nc.tensor.matmul(pp[:], wk[:, kc, :],
                                     xbT[kc // 4][:, kc % 4, :],
                                     start=(kc == 0), stop=(kc == 7))
                ks = p1.tile([128, 512], MM, tag="kstage")
                nc.scalar.activation(ks[:], pp[:], AF.Identity,
                                     bias=bias["bk"][:, f:f + 1])
                nc.gpsimd.dma_start(
                    kt_dram[f * 128:(f + 1) * 128, rb * 512:(rb + 1) * 512],
                    ks[:])

            # v: rows rb*512..+512, all 1024 v-cols -> v_aug tiles
            for nb in range(2):
                wv = [p1wv.tile([128, 512], MM, tag="wv", name=f"wv{i}")
                  for i in range(8)]
                for kc in range(8):
                    nc.sync.dma_start(
                        wv[kc][:].rearrange("p (m n) -> p m n", m=4),
                        _r(wqkv_t[kc, 16 + nb * 4:16 + (nb + 1) * 4, :, :]
                           .rearrange("m p n -> p m n")))
                for rt in range(4):
                    pp = ps1p.tile([128, 512], F32, tag="proj")
                    for kc in range(8):
                        nc.tensor.matmul(
                            pp[:],
                            xbT[kc // 4][:, kc % 4, rt * 128:(rt + 1) * 128],
                            wv[kc][:], start=(kc == 0), stop=(kc == 7))
                    nc.vector.tensor_copy(
                        v_aug[rb * 4 + rt][:, nb * 8:(nb + 1) * 8, 0:D],
                        pp[:].rearrange("p (h d) -> p h d", d=D))


def _phase2(nc, tc, AF, OP, t_, bias, ones, v_aug, qT, xqT, x2T):
    """Attention per head, transposed flow; writes x2T = xq + attn_out (fp32).

    kv rows arrive block-permuted (8 blocks of 256): slot 3 = the A-chunk's
    diagonal block, slot 7 = the B-chunk's. Prefix blocks need only a
    per-block additive gate (0 / -1e9), folded into the Exp activation's
    bias, so only diagonal blocks pay a DVE mask-add (constant tri mask).
    AV matmuls and the normalize/residual chain are software-pipelined one
    step behind so the in-order PE queue never stalls on DVE/ACT latency."""
    gates, maskd, kt_dram = t_["gates"], t_["maskd"], t_["kt_dram"]
    with tc.tile_pool(name="p2m", bufs=1) as p2m, \
         tc.tile_pool(name="p2k", bufs=2) as p2k, \
         tc.tile_pool(name="p2w", bufs=6) as p2w, \
         tc.tile_pool(name="ps2s", bufs=3, space="PSUM") as ps2s, \
         tc.tile_pool(name="ps2o", bufs=3, space="PSUM") as ps2o, \
         tc.tile_pool(name="ps2b", bufs=2, space="PSUM") as ps2b:

        # gates[:, 2*s + (0:A,1:B)] : bias column for slot s
        gt = p2m.tile([128, 16], F32, tag="gt")
        nc.sync.dma_start(gt[:], gates)
        md = p2m.tile([128, 2, CH], F32, tag="md")
        nc.sync.dma_start(md[:], maskd.rearrange("(c p) q -> p c q", p=128))

        # odd-head residual operands shifted down to partitions 0:64
        xq_lo = [p2m.tile([128, 2 * CH], F32, tag=f"xql{f}", name=f"xq_lo{f}")
                 for f in range(8)]
        for f in range(8):
            nc.gpsimd.dma_start(xq_lo[f][0:D, :], xqT[f][D:128, :])

        pending = []          # AV work deferred one block-iteration
        fin_q = []            # finalize work deferred one full (hp,qoff) unit

        def flush():
            for fn in pending:
                fn()
            pending.clear()
            while len(fin_q) > 1:
                fin_q.pop(0)()

        for f in range(8):
            kth = p2k.tile([128, T], MM, tag="kth")
            nc.sync.dma_start(kth[:], kt_dram[f * 128:(f + 1) * 128, :])
            x2lo = p2w.tile([128, 2 * CH], F32, tag="x2lo", name=f"x2lo{f}",
                            bufs=2)
            for hp in range(2):
                h = 2 * f + hp
                po = 64 * hp
                qh = qT[f][po:po + D, :]
                bv_h = bias["bv"] if hp == 0 else bias["bvo"]
                for (qi, qoff, nblk) in ((0, 0, 4), (1, CH, 8)):
                    diag = nblk - 1
                    acc = ps2o.tile([128, CH], F32, tag="po")
                    for blk in range(nblk):
                        ps = ps2s.tile([128, 2, CH], F32, tag="ps")
                        for s in range(2):
                            c = 2 * blk + s
                            nc.tensor.matmul(
                                ps[:, s, :],
                                kth[po:po + D, c * 128:(c + 1) * 128],
                                qh[:, qoff:qoff + CH], start=True, stop=True)
                        ex = p2w.tile([128, 2, CH], MM, tag="ex", bufs=4)
                        if blk == diag:
                            sm = p2w.tile([128, 2, CH], F32, tag="sm", bufs=2)
                            nc.vector.tensor_tensor(out=sm[:], in0=ps[:],
                                                    in1=md[:], op=OP.add)
                            nc.scalar.activation(ex[:], sm[:], AF.Exp)
                        else:
                            nc.scalar.activation(
                                ex[:], ps[:], AF.Exp,
                                bias=gt[:, 2 * blk + qi:2 * blk + qi + 1])
                        flush()

                        def mk_avs(ex=ex, blk=blk, h=h, acc=acc, nblk=nblk):
                            def go():
                                for s in range(2):
                                    c = 2 * blk + s
                                    nc.tensor.matmul(
                                        acc[0:D + 1, :], v_aug[c][:, h, :],
                                        ex[:, s, :], start=(c == 0),
                                        stop=(c == 2 * nblk - 1))
                            return go
                        pending.append(mk_avs())

                    def mk_fin(acc=acc, hp=hp, qoff=qoff, f=f, bv_h=bv_h,
                               x2lo=x2lo):
                        def go():
                            rec = p2w.tile([128, CH], F32, tag="rec", bufs=2)
                            nc.vector.reciprocal(rec[D:D + 1, :],
                                                 acc[D:D + 1, :])
                            pb = ps2b.tile([128, CH], F32, tag="pb")
                            nc.tensor.matmul(pb[0:D, :], ones[D:D + 1, :],
                                             rec[D:D + 1, :], start=True,
                                             stop=True)
                            sb = p2w.tile([128, CH], F32, tag="sb", bufs=2)
                            nc.scalar.copy(sb[0:D, :], pb[0:D, :])
                            tt = p2w.tile([128, CH], F32, tag="tt", bufs=2)
                            nc.vector.tensor_tensor(
                                out=tt[0:D, :], in0=acc[0:D, :],
                                in1=sb[0:D, :], op=OP.mult)
                            nc.vector.tensor_scalar_add(
                                tt[0:D, :], tt[0:D, :], bv_h[0:D, f:f + 1])
                            if hp == 0:
                                nc.vector.tensor_tensor(
                                    out=x2T[f][0:D, qoff:qoff + CH],
                                    in0=tt[0:D, :],
                                    in1=xqT[f][0:D, qoff:qoff + CH],
                                    op=OP.add)
                            else:
                                nc.vector.tensor_tensor(
                                    out=x2lo[0:D, qoff:qoff + CH],
                                    in0=tt[0:D, :],
                                    in1=xq_lo[f][0:D, qoff:qoff + CH],
                                    op=OP.add)
                                if qoff == CH:
                                    nc.gpsimd.dma_start(x2T[f][D:128, :],
                                                        x2lo[0:D, :])
                        return go
                    fin_q.append(mk_fin())
        flush()
        for fn in fin_q:
            fn()
        fin_q.clear()


def _phase3(nc, tc, AF, OP, t_, bias, ident, x2T, out):
    """MLP (transposed) + residual + transpose back to natural layout."""
    w1_t, w2_t = t_["w1_t"], t_["w2_t"]
    with tc.tile_pool(name="p3h", bufs=8) as p3h, \
         tc.tile_pool(name="p3w1", bufs=4) as p3w1, \
         tc.tile_pool(name="p3w2", bufs=2) as p3w2, \
         tc.tile_pool(name="p3s", bufs=2) as p3s, \
         tc.tile_pool(name="p3y", bufs=1) as p3y, \
         tc.tile_pool(name="ps3p", bufs=4, space="PSUM") as ps3p, \
         tc.tile_pool(name="ps3t", bufs=2, space="PSUM") as ps3t:

        if USE_F32R:
            x2r = [p3y.tile([128, 2 * CH], F32R, tag=f"x2r{f}",
                           name=f"x2r{f}") for f in range(8)]
            for f in range(8):
                nc.vector.tensor_copy(x2r[f][:], x2T[f][:])
        else:
            x2r = x2T

        hT = [p3h.tile([128, 4, 2 * CH], MM, tag="ht", name=f"hT{i}")
              for i in range(8)]
        for m in range(32):
            w1s = p3w1.tile([128, 8, 128], MM, tag="w1s")
            nc.sync.dma_start(
                w1s[:], _r(w1_t[m, :, :, :].rearrange("k p n -> p k n")))
            pp = ps3p.tile([128, 2 * CH], F32, tag="proj")
            for kc in range(8):
                nc.tensor.matmul(pp[:], w1s[:, kc, :], x2r[kc][:],
                                 start=(kc == 0), stop=(kc == 7))
            nc.scalar.activation(hT[m // 4][:, m % 4, :], pp[:], AF.Relu,
                                 bias=bias["b1"][:, m:m + 1])

        yt = []
        for mo in range(8):
            w2s = p3w2.tile([128, 32, 128], MM, tag="w2s")
            nc.sync.dma_start(
                w2s[:], _r(w2_t[mo, :, :, :].rearrange("k p n -> p k n")))
            pp = ps3p.tile([128, 2 * CH], F32, tag="proj")
            for kc in range(32):
                nc.tensor.matmul(pp[:], w2s[:, kc, :], hT[kc // 4][:, kc % 4, :],
                                 start=(kc == 0), stop=(kc == 31))
            ys = p3y.tile([128, 2 * CH], F32, tag=f"yt{mo}", name=f"ys{mo}")
            nc.scalar.activation(ys[:], pp[:], AF.Identity,
                                 bias=bias["b2"][:, mo:mo + 1])
            nc.vector.tensor_tensor(out=ys[:], in0=ys[:], in1=x2T[mo][:],
                                    op=OP.add)
            yt.append(ys)

        # transpose back: out[rows, feats]
        for rt in range(4):
            onat = p3s.tile([128, 1024], F32, tag="onat")
            for mo in range(8):
                pt = ps3t.tile([128, 128], F32, tag="tp")
                nc.tensor.transpose(pt[:], yt[mo][:, rt * 128:(rt + 1) * 128],
                                    ident[:])
                nc.scalar.copy(onat[:, mo * 128:(mo + 1) * 128], pt[:])
            nc.gpsimd.dma_start(out[rt * 128:(rt + 1) * 128, :], onat[:])


# --------------------------------------------------------------- host driver
def _install_ntff_hook():
    """The container's antenv stub lacks axon_hooks; provide it so
    run_bass_kernel_spmd(trace=True) can capture NTFF profiles via libaxon."""
    import types

    try:
        import antenv.axon_hooks  # noqa: F401
        return
    except ImportError:
        pass
    holder = {"h": None}
    mod = types.ModuleType("antenv.axon_hooks")
    mod.set_axon_ntff_profile_hook = lambda h: holder.__setitem__("h", h)
    mod.get_axon_ntff_profile_hook = lambda: holder["h"]
    sys.modules["antenv.axon_hooks"] = mod
    import antenv

    antenv.axon_hooks = mod
    if "/root/.axon_site" not in sys.path:
        sys.path.insert(0, "/root/.axon_site")
    from trn_agent_boot.trn_boot import _ntff_profile_via_ctypes

    so = "/opt/axon/libaxon_pjrt.so"
    if os.path.exists(so):
        mod.set_axon_ntff_profile_hook(_ntff_profile_via_ctypes(so))


def _get_program():
    key = ("v1", USE_F32R)
    if key not in _prog_cache:
        nc = _build_program()
        _legalize_waits(nc)
        _prog_cache[key] = nc
    return _prog_cache[key]


def _prep_shared(Wqkv, W1, W2, bqkv, b1, b2):
    bv_ = bqkv[2 * N:]
    bvo = np.zeros(N, np.float32)
    bvo.reshape(8, 128)[:, 0:64] = bv_.reshape(8, 128)[:, 64:128]
    wqkv_t = np.ascontiguousarray(
        Wqkv.reshape(8, 128, 24, 128).transpose(0, 2, 1, 3))
    w1_t = np.ascontiguousarray(
        W1.reshape(8, 128, 32, 128).transpose(2, 0, 1, 3))
    w2_t = np.ascontiguousarray(
        W2.reshape(32, 128, 8, 128).transpose(2, 0, 1, 3))
    return {
        "wqkv_t": wqkv_t,
        "w1_t": w1_t, "w2_t": w2_t,
        "bqs": np.ascontiguousarray(bqkv[:N] * 0.125),
        "bk": np.ascontiguousarray(bqkv[N:2 * N]),
        "bv": np.ascontiguousarray(bv_),
        "bvo": bvo,
        "b1": np.ascontiguousarray(b1), "b2": np.ascontiguousarray(b2),
    }


def _core_chunks(c):
    b, j = c // 4, c % 4
    return b, j, 7 - j


def _slot_blocks(j):
    # slot order of the 8 kv row-blocks: slot 3 = A diag (block j),
    # slot 7 = B diag (block 7-j), others ascending.
    other = [b for b in range(8) if b not in (j, 7 - j)]
    return [other[0], other[1], other[2], j, other[3], other[4], other[5],
            7 - j]


def _make_gates(j):
    slots = _slot_blocks(j)
    g = np.full((128, 16), -1e9, np.float32)
    for s in range(8):
        if s != 3 and slots[s] < j:
            g[:, 2 * s] = 0.0          # allowed for A
        if s != 7 and slots[s] < 7 - j:
            g[:, 2 * s + 1] = 0.0      # allowed for B
    return g


_MASKD = np.where(np.arange(256)[:, None] <= np.arange(CH)[None, :],
                  0.0, -1e9).astype(np.float32)


def kernel(x, Wqkv, bqkv, W1, b1, W2, b2, _trace=False):
    x = np.asarray(x, dtype=np.float32)
    shared = _prep_shared(np.asarray(Wqkv, np.float32),
                          np.asarray(W1, np.float32),
                          np.asarray(W2, np.float32),
                          np.asarray(bqkv, np.float32),
                          np.asarray(b1, np.float32),
                          np.asarray(b2, np.float32))
    in_maps = []
    for c in range(NCORES):
        b, j, jb = _core_chunks(c)
        xqc = np.concatenate(
            [x[b, j * CH:(j + 1) * CH], x[b, jb * CH:(jb + 1) * CH]], axis=0)
        xbp = x[b].reshape(8, CH, N)[_slot_blocks(j)].reshape(T, N)
        in_maps.append({
            **shared,
            "xb": np.ascontiguousarray(xbp),
            "xq": np.ascontiguousarray(xqc),
            "gates": _make_gates(j), "maskd": _MASKD,
        })

    nc = _get_program()
    if _trace:
        _install_ntff_hook()
    res = run_bass_kernel_spmd(nc, in_maps, list(range(NCORES)), trace=_trace)

    outf = np.empty((B, T, N), dtype=np.float32)
    for c in range(NCORES):
        b, j, jb = _core_chunks(c)
        o = res.results[c]["out"]
        outf[b, j * CH:(j + 1) * CH] = o[:CH]
        outf[b, jb * CH:(jb + 1) * CH] = o[CH:]
    if _trace:
        kernel.last_results = res
    return outf
